# revision 4
# baseline (speedup 1.0000x reference)
"""CrossConv2d (concat -> 3x3 conv -> BN -> +skip -> ReLU) on 8 Trainium2 cores.

Data-parallel over the fused (b*s)=32 batch axis: 4 images per core.
Per-core Bass/Tile kernel:
  - channels (64 u + 64 v = 128) live on SBUF partitions
  - image staged in strips of 32 output rows (+1 halo row each side),
    width padded 128 -> 130 with zero columns so the 3x3 conv is 9
    shifted matmuls (lhsT = W[tap] as [C_in, C_out], fp32r) accumulating
    into PSUM over 512-pixel chunks
  - BN scale folded into the conv weights host-side; BN shift applied as
    the ScalarE Relu bias; skip-add is one VectorE add (in-place in PSUM)
"""

import numpy as np

import concourse.bacc as bacc
import concourse.mybir as mybir
from concourse import tile
from concourse.bass_utils import run_bass_kernel_spmd

EPS = 1e-5

B, S, C1, C2, H, W = 4, 8, 64, 64, 128, 128
CC = C1 + C2  # 128 concat channels = out channels = partition count
N_CORES = 8
IMG_PER_CORE = (B * S) // N_CORES  # 4
STRIP = 32                 # output rows per strip
NSTRIPS = H // STRIP
WP = W + 2                 # padded width
HALO = STRIP + 2           # input rows staged per strip
CHUNK = 512                # one PSUM bank of fp32

F32 = mybir.dt.float32
MM_DT = mybir.dt.float32r  # full-rate fp32 matmul mode

_CACHE = {}


def _build_program():
    nc = bacc.Bacc(
        "TRN2", target_bir_lowering=False, debug=False, num_devices=N_CORES
    )
    u_d = nc.dram_tensor("u", [C1, H, W], F32, kind="ExternalInput")
    v_d = nc.dram_tensor("v", [IMG_PER_CORE, C2, H, W], F32, kind="ExternalInput")
    w_d = nc.dram_tensor("w", [CC, 9 * CC], F32, kind="ExternalInput")
    sh_d = nc.dram_tensor("shift", [CC, 1], F32, kind="ExternalInput")
    o_d = nc.dram_tensor("o", [IMG_PER_CORE, CC, H, W], F32, kind="ExternalOutput")

    with tile.TileContext(nc) as tc:
        with (
            tc.tile_pool(name="consts", bufs=1) as cpool,
            tc.tile_pool(name="xs", bufs=3) as xpool,
            tc.tile_pool(name="xr", bufs=3) as xrpool,
            tc.tile_pool(name="ostrip", bufs=2) as opool,
            tc.tile_pool(name="psum", bufs=8, space="PSUM") as ppool,
        ):
            w_sb = cpool.tile([CC, 9 * CC], F32)
            nc.sync.dma_start(w_sb[:], w_d[:])
            sh_sb = cpool.tile([CC, 1], F32)
            nc.sync.dma_start(sh_sb[:], sh_d[:])
            # PE consumes fp32r, which must be produced by an on-chip
            # rounding op (walrus checkMatmultFP32r)
            w_r = cpool.tile([CC, 9 * CC], MM_DT)
            nc.scalar.copy(w_r[:], w_sb[:])

            for img in range(IMG_PER_CORE):
                for s in range(NSTRIPS):
                    xs = xpool.tile([CC, HALO * WP], F32)
                    xs3 = xs[:].rearrange("p (r w) -> p r w", w=WP)
                    # zero the whole staging tile (covers pad columns + halo
                    # rows at image boundaries), then DMA the interior
                    nc.gpsimd.memset(xs[:], 0.0)
                    r0 = s * STRIP - 1          # image row of strip row 0
                    r1 = s * STRIP + STRIP + 1  # exclusive
                    srow = 0
                    if r0 < 0:
                        r0, srow = 0, 1
                    if r1 > H:
                        r1 = H
                    nr = r1 - r0
                    nc.sync.dma_start(
                        xs3[0:C1, srow : srow + nr, 1 : 1 + W], u_d[:, r0:r1, :]
                    )
                    nc.sync.dma_start(
                        xs3[C1:CC, srow : srow + nr, 1 : 1 + W],
                        v_d[img, :, r0:r1, :],
                    )
                    xr = xrpool.tile([CC, HALO * WP], MM_DT)
                    nc.scalar.copy(xr[:], xs[:])

                    ostrip = opool.tile([CC, STRIP * WP], F32)
                    q0, q1 = 1, STRIP * WP - 1
                    for qc0 in range(q0, q1, CHUNK):
                        qc1 = min(qc0 + CHUNK, q1)
                        n = qc1 - qc0
                        ps = ppool.tile([CC, CHUNK], F32)
                        for t in range(9):
                            dy, dx = t // 3 - 1, t % 3 - 1
                            off = (1 + dy) * WP + dx
                            nc.tensor.matmul(
                                ps[:, 0:n],
                                w_r[:, t * CC : (t + 1) * CC],
                                xr[:, qc0 + off : qc1 + off],
                                start=(t == 0),
                                stop=(t == 8),
                            )
                        # skip-add: out flat index q reads input flat q + WP
                        nc.vector.tensor_add(
                            ps[:, 0:n], ps[:, 0:n], xs[:, qc0 + WP : qc1 + WP]
                        )
                        nc.scalar.activation(
                            ostrip[:, qc0:qc1],
                            ps[:, 0:n],
                            mybir.ActivationFunctionType.Relu,
                            bias=sh_sb[:],
                            scale=1.0,
                        )
                    o3 = ostrip[:].rearrange("p (r w) -> p r w", w=WP)
                    nc.sync.dma_start(
                        o_d[img, :, s * STRIP : (s + 1) * STRIP, :],
                        o3[:, :, 1 : 1 + W],
                    )
    nc.compile()
    return nc


def _get_program():
    if "nc" not in _CACHE:
        _CACHE["nc"] = _build_program()
    return _CACHE["nc"]


def _prep_inputs(u, v, conv_w, bn_gamma, bn_beta, bn_mean, bn_var):
    u = np.asarray(u, dtype=np.float32)
    v = np.asarray(v, dtype=np.float32)
    conv_w = np.asarray(conv_w, dtype=np.float32)
    bn_gamma = np.asarray(bn_gamma, dtype=np.float32)
    bn_beta = np.asarray(bn_beta, dtype=np.float32)
    bn_mean = np.asarray(bn_mean, dtype=np.float32)
    bn_var = np.asarray(bn_var, dtype=np.float32)

    scale = bn_gamma / np.sqrt(bn_var + EPS)
    shift = (bn_beta - bn_mean * scale).astype(np.float32).reshape(CC, 1)
    wsc = (conv_w * scale[:, None, None, None]).astype(np.float32)
    # lhsT layout per tap t = ky*3+kx: w_host[i, t*CC + o] = wsc[o, i, ky, kx]
    w_host = np.ascontiguousarray(
        wsc.transpose(1, 2, 3, 0).reshape(CC, 9 * CC)
    )

    in_maps = []
    for m in range(N_CORES):
        b = m // 2
        s0 = (m % 2) * IMG_PER_CORE
        in_maps.append(
            {
                "u": np.ascontiguousarray(u[b, 0]),
                "v": np.ascontiguousarray(v[b, s0 : s0 + IMG_PER_CORE]),
                "w": w_host,
                "shift": shift,
            }
        )
    return in_maps


def _run(inputs, trace=False):
    nc = _get_program()
    in_maps = _prep_inputs(**inputs)
    res = run_bass_kernel_spmd(
        nc, in_maps, list(range(N_CORES)), trace=trace
    )
    out = np.empty((B, 1, S, CC, H, W), np.float32)
    for m in range(N_CORES):
        b = m // 2
        s0 = (m % 2) * IMG_PER_CORE
        out[b, 0, s0 : s0 + IMG_PER_CORE] = res.results[m]["o"]
    return out, res


def kernel(u, v, conv_w, bn_gamma, bn_beta, bn_mean, bn_var):
    out, _ = _run(
        dict(
            u=u,
            v=v,
            conv_w=conv_w,
            bn_gamma=bn_gamma,
            bn_beta=bn_beta,
            bn_mean=bn_mean,
            bn_var=bn_var,
        )
    )
    return out


# revision 6
# speedup vs baseline: 82.3586x; 82.3586x over previous
"""CrossConv2d (concat -> 3x3 conv -> BN -> +skip -> ReLU) on 8 Trainium2 cores.

Data-parallel over the fused (b*s)=32 batch axis: 4 images per core.
Per-core Bass/Tile kernel:
  - channels (64 u + 64 v = 128) live on SBUF partitions
  - image staged in strips of 32 output rows (+1 halo row each side),
    width padded 128 -> 130 with zero columns so the 3x3 conv is 9
    shifted matmuls (lhsT = W[tap] as [C_in, C_out], fp32r) accumulating
    into PSUM over 512-pixel chunks
  - BN scale folded into the conv weights host-side; BN shift applied as
    the ScalarE Relu bias; skip-add is one VectorE add (in-place in PSUM)
"""

import numpy as np

import concourse.bacc as bacc
import concourse.mybir as mybir
from concourse import tile
from concourse.bass_utils import run_bass_kernel_spmd

EPS = 1e-5

B, S, C1, C2, H, W = 4, 8, 64, 64, 128, 128
CC = C1 + C2  # 128 concat channels = out channels = partition count
N_CORES = 8
IMG_PER_CORE = (B * S) // N_CORES  # 4
STRIP = 32                 # output rows per strip
NSTRIPS = H // STRIP
WP = W + 2                 # padded width
HALO = STRIP + 2           # input rows staged per strip
CHUNK = 462                # pixels per PSUM chunk (9 x 462 = 4158; all >=256 keeps fp32r at full rate)

F32 = mybir.dt.float32
MM_DT = mybir.dt.float32r  # full-rate fp32 matmul mode

_CACHE = {}


def _build_program():
    nc = bacc.Bacc(
        "TRN2", target_bir_lowering=False, debug=False, num_devices=N_CORES
    )
    u_d = nc.dram_tensor("u", [C1, H, W], F32, kind="ExternalInput")
    v_d = nc.dram_tensor("v", [IMG_PER_CORE, C2, H, W], F32, kind="ExternalInput")
    w_d = nc.dram_tensor("w", [CC, 9 * CC], F32, kind="ExternalInput")
    sh_d = nc.dram_tensor("shift", [CC, 1], F32, kind="ExternalInput")
    o_d = nc.dram_tensor("o", [IMG_PER_CORE, CC, H, W], F32, kind="ExternalOutput")

    with tile.TileContext(nc) as tc:
        with (
            tc.tile_pool(name="consts", bufs=1) as cpool,
            tc.tile_pool(name="xs", bufs=3) as xpool,
            tc.tile_pool(name="xr", bufs=3) as xrpool,
            tc.tile_pool(name="ostrip", bufs=3) as opool,
            tc.tile_pool(name="psum", bufs=8, space="PSUM") as ppool,
        ):
            w_sb = cpool.tile([CC, 9 * CC], F32)
            nc.sync.dma_start(w_sb[:], w_d[:])
            sh_sb = cpool.tile([CC, 1], F32)
            nc.sync.dma_start(sh_sb[:], sh_d[:])
            # PE consumes fp32r, which must be produced by an on-chip
            # rounding op (walrus checkMatmultFP32r)
            w_r = cpool.tile([CC, 9 * CC], MM_DT)
            nc.scalar.copy(w_r[:], w_sb[:])

            for img in range(IMG_PER_CORE):
                for s in range(NSTRIPS):
                    xs = xpool.tile([CC, HALO * WP], F32)
                    xs3 = xs[:].rearrange("p (r w) -> p r w", w=WP)
                    # zero only the pad columns (strided) and boundary halo
                    # rows; DMA fills the interior
                    nc.vector.memset(xs3[:, :, 0:1], 0.0)
                    nc.vector.memset(xs3[:, :, WP - 1 : WP], 0.0)
                    r0 = s * STRIP - 1          # image row of strip row 0
                    r1 = s * STRIP + STRIP + 1  # exclusive
                    srow = 0
                    if r0 < 0:
                        r0, srow = 0, 1
                        nc.vector.memset(xs3[:, 0, 1 : 1 + W], 0.0)
                    if r1 > H:
                        r1 = H
                        nc.vector.memset(xs3[:, HALO - 1, 1 : 1 + W], 0.0)
                    nr = r1 - r0
                    nc.sync.dma_start(
                        xs3[0:C1, srow : srow + nr, 1 : 1 + W], u_d[:, r0:r1, :]
                    )
                    nc.sync.dma_start(
                        xs3[C1:CC, srow : srow + nr, 1 : 1 + W],
                        v_d[img, :, r0:r1, :],
                    )
                    xr = xrpool.tile([CC, HALO * WP], MM_DT)
                    nc.gpsimd.tensor_copy(xr[:], xs[:])

                    ostrip = opool.tile([CC, STRIP * WP], F32)
                    q0, q1 = 1, STRIP * WP - 1
                    for qc0 in range(q0, q1, CHUNK):
                        qc1 = min(qc0 + CHUNK, q1)
                        n = qc1 - qc0
                        ps = ppool.tile([CC, CHUNK], F32)
                        for t in range(9):
                            dy, dx = t // 3 - 1, t % 3 - 1
                            off = (1 + dy) * WP + dx
                            nc.tensor.matmul(
                                ps[:, 0:n],
                                w_r[:, t * CC : (t + 1) * CC],
                                xr[:, qc0 + off : qc1 + off],
                                start=(t == 0),
                                stop=(t == 8),
                            )
                        # skip-add: out flat index q reads input flat q + WP
                        nc.vector.tensor_add(
                            ps[:, 0:n], ps[:, 0:n], xs[:, qc0 + WP : qc1 + WP]
                        )
                        nc.scalar.activation(
                            ostrip[:, qc0:qc1],
                            ps[:, 0:n],
                            mybir.ActivationFunctionType.Relu,
                            bias=sh_sb[:],
                            scale=1.0,
                        )
                    o3 = ostrip[:].rearrange("p (r w) -> p r w", w=WP)
                    nc.sync.dma_start(
                        o_d[img, :, s * STRIP : (s + 1) * STRIP, :],
                        o3[:, :, 1 : 1 + W],
                    )
    nc.compile()
    return nc


def _get_program():
    if "nc" not in _CACHE:
        _CACHE["nc"] = _build_program()
    return _CACHE["nc"]


def _prep_inputs(u, v, conv_w, bn_gamma, bn_beta, bn_mean, bn_var):
    u = np.asarray(u, dtype=np.float32)
    v = np.asarray(v, dtype=np.float32)
    conv_w = np.asarray(conv_w, dtype=np.float32)
    bn_gamma = np.asarray(bn_gamma, dtype=np.float32)
    bn_beta = np.asarray(bn_beta, dtype=np.float32)
    bn_mean = np.asarray(bn_mean, dtype=np.float32)
    bn_var = np.asarray(bn_var, dtype=np.float32)

    scale = bn_gamma / np.sqrt(bn_var + EPS)
    shift = (bn_beta - bn_mean * scale).astype(np.float32).reshape(CC, 1)
    wsc = (conv_w * scale[:, None, None, None]).astype(np.float32)
    # lhsT layout per tap t = ky*3+kx: w_host[i, t*CC + o] = wsc[o, i, ky, kx]
    w_host = np.ascontiguousarray(
        wsc.transpose(1, 2, 3, 0).reshape(CC, 9 * CC)
    )

    in_maps = []
    for m in range(N_CORES):
        b = m // 2
        s0 = (m % 2) * IMG_PER_CORE
        in_maps.append(
            {
                "u": np.ascontiguousarray(u[b, 0]),
                "v": np.ascontiguousarray(v[b, s0 : s0 + IMG_PER_CORE]),
                "w": w_host,
                "shift": shift,
            }
        )
    return in_maps


def _run(inputs, trace=False):
    nc = _get_program()
    in_maps = _prep_inputs(**inputs)
    res = run_bass_kernel_spmd(
        nc, in_maps, list(range(N_CORES)), trace=trace
    )
    out = np.empty((B, 1, S, CC, H, W), np.float32)
    for m in range(N_CORES):
        b = m // 2
        s0 = (m % 2) * IMG_PER_CORE
        out[b, 0, s0 : s0 + IMG_PER_CORE] = res.results[m]["o"]
    return out, res


def kernel(u, v, conv_w, bn_gamma, bn_beta, bn_mean, bn_var):
    out, _ = _run(
        dict(
            u=u,
            v=v,
            conv_w=conv_w,
            bn_gamma=bn_gamma,
            bn_beta=bn_beta,
            bn_mean=bn_mean,
            bn_var=bn_var,
        )
    )
    return out


# revision 8
# speedup vs baseline: 103.2210x; 1.2533x over previous
"""CrossConv2d (concat -> 3x3 conv -> BN -> +skip -> ReLU) on 8 Trainium2 cores.

Data-parallel over the fused (b*s)=32 batch axis: 4 images per core.
Per-core Bass/Tile kernel:
  - channels (64 u + 64 v = 128) live on SBUF partitions
  - image staged in strips of 32 output rows (+1 halo row each side),
    width padded 128 -> 130 with zero columns so the 3x3 conv is 9
    shifted matmuls (lhsT = W[tap] as [C_in, C_out], fp32r) accumulating
    into PSUM over 512-pixel chunks
  - BN scale folded into the conv weights host-side; BN shift applied as
    the ScalarE Relu bias; skip-add is one VectorE add (in-place in PSUM)
"""

import numpy as np

import concourse.bacc as bacc
import concourse.mybir as mybir
from concourse import tile
from concourse.bass_utils import run_bass_kernel_spmd

EPS = 1e-5

B, S, C1, C2, H, W = 4, 8, 64, 64, 128, 128
CC = C1 + C2  # 128 concat channels = out channels = partition count
N_CORES = 8
IMG_PER_CORE = (B * S) // N_CORES  # 4
STRIP = 32                 # output rows per strip
NSTRIPS = H // STRIP
WP = W + 2                 # padded width
HALO = STRIP + 2           # input rows staged per strip
CHUNK = 462                # pixels per PSUM chunk (9 x 462 = 4158; all >=256 keeps fp32r at full rate)

F32 = mybir.dt.float32
MM_DT = mybir.dt.float32r  # full-rate fp32 matmul mode

_CACHE = {}


def _build_program():
    nc = bacc.Bacc(
        "TRN2", target_bir_lowering=False, debug=False, num_devices=N_CORES
    )
    u_d = nc.dram_tensor("u", [C1, H, W], F32, kind="ExternalInput")
    v_d = nc.dram_tensor("v", [IMG_PER_CORE, C2, H, W], F32, kind="ExternalInput")
    w_d = nc.dram_tensor("w", [CC, 9 * CC], F32, kind="ExternalInput")
    sh_d = nc.dram_tensor("shift", [CC, 1], F32, kind="ExternalInput")
    o_d = nc.dram_tensor("o", [IMG_PER_CORE, CC, H, W], F32, kind="ExternalOutput")

    with tile.TileContext(nc) as tc:
        with (
            tc.tile_pool(name="consts", bufs=1) as cpool,
            tc.tile_pool(name="xs", bufs=3) as xpool,
            tc.tile_pool(name="xr", bufs=3) as xrpool,
            tc.tile_pool(name="ostrip", bufs=3) as opool,
            tc.tile_pool(name="psum", bufs=8, space="PSUM") as ppool,
        ):
            w_sb = cpool.tile([CC, 9 * CC], F32)
            nc.sync.dma_start(w_sb[:], w_d[:])
            sh_sb = cpool.tile([CC, 1], F32)
            nc.sync.dma_start(sh_sb[:], sh_d[:])
            # PE consumes fp32r, which must be produced by an on-chip
            # rounding op (walrus checkMatmultFP32r)
            w_r = cpool.tile([CC, 9 * CC], MM_DT)
            nc.scalar.copy(w_r[:], w_sb[:])

            for img in range(IMG_PER_CORE):
                for s in range(NSTRIPS):
                    xs = xpool.tile([CC, HALO * WP], F32)
                    xs3 = xs[:].rearrange("p (r w) -> p r w", w=WP)
                    # zero only the pad columns (strided) and boundary halo
                    # rows; DMA fills the interior
                    nc.vector.memset(xs3[:, :, 0:1], 0.0)
                    nc.vector.memset(xs3[:, :, WP - 1 : WP], 0.0)
                    r0 = s * STRIP - 1          # image row of strip row 0
                    r1 = s * STRIP + STRIP + 1  # exclusive
                    srow = 0
                    if r0 < 0:
                        r0, srow = 0, 1
                        nc.vector.memset(xs3[:, 0, 1 : 1 + W], 0.0)
                    if r1 > H:
                        r1 = H
                        nc.vector.memset(xs3[:, HALO - 1, 1 : 1 + W], 0.0)
                    nr = r1 - r0
                    nc.sync.dma_start(
                        xs3[0:C1, srow : srow + nr, 1 : 1 + W], u_d[:, r0:r1, :]
                    )
                    nc.sync.dma_start(
                        xs3[C1:CC, srow : srow + nr, 1 : 1 + W],
                        v_d[img, :, r0:r1, :],
                    )
                    # fp32 -> fp32r rounding pass, split across ACT and DVE
                    xr = xrpool.tile([CC, HALO * WP], MM_DT)
                    half = (HALO * WP) // 2
                    nc.scalar.copy(xr[:, 0:half], xs[:, 0:half])
                    nc.vector.tensor_copy(xr[:, half:], xs[:, half:])

                    ostrip = opool.tile([CC, STRIP * WP], F32)
                    q0, q1 = 1, STRIP * WP - 1
                    chunks = [
                        (qc0, min(qc0 + CHUNK, q1))
                        for qc0 in range(q0, q1, CHUNK)
                    ]
                    # taps-outer over groups of 3 chunks: each weight load
                    # feeds 3 matmuls
                    pss = {}
                    for g0 in range(0, len(chunks), 3):
                        grp = chunks[g0 : g0 + 3]
                        for c, _ in grp:
                            ps_g = ppool.tile([CC, CHUNK], F32, tag="ps")
                            pss[c] = ps_g
                        for t in range(9):
                            dy, dx = t // 3 - 1, t % 3 - 1
                            off = (1 + dy) * WP + dx
                            for qc0, qc1 in grp:
                                nc.tensor.matmul(
                                    pss[qc0][:, 0 : qc1 - qc0],
                                    w_r[:, t * CC : (t + 1) * CC],
                                    xr[:, qc0 + off : qc1 + off],
                                    start=(t == 0),
                                    stop=(t == 8),
                                )
                        for qc0, qc1 in grp:
                            n = qc1 - qc0
                            ps = pss[qc0]
                            # skip-add: out flat q reads input flat q + WP
                            nc.vector.tensor_add(
                                ps[:, 0:n], ps[:, 0:n], xs[:, qc0 + WP : qc1 + WP]
                            )
                            nc.scalar.activation(
                                ostrip[:, qc0:qc1],
                                ps[:, 0:n],
                                mybir.ActivationFunctionType.Relu,
                                bias=sh_sb[:],
                                scale=1.0,
                            )
                    o3 = ostrip[:].rearrange("p (r w) -> p r w", w=WP)
                    nc.sync.dma_start(
                        o_d[img, :, s * STRIP : (s + 1) * STRIP, :],
                        o3[:, :, 1 : 1 + W],
                    )
    nc.compile()
    return nc


def _get_program():
    if "nc" not in _CACHE:
        _CACHE["nc"] = _build_program()
    return _CACHE["nc"]


def _prep_inputs(u, v, conv_w, bn_gamma, bn_beta, bn_mean, bn_var):
    u = np.asarray(u, dtype=np.float32)
    v = np.asarray(v, dtype=np.float32)
    conv_w = np.asarray(conv_w, dtype=np.float32)
    bn_gamma = np.asarray(bn_gamma, dtype=np.float32)
    bn_beta = np.asarray(bn_beta, dtype=np.float32)
    bn_mean = np.asarray(bn_mean, dtype=np.float32)
    bn_var = np.asarray(bn_var, dtype=np.float32)

    scale = bn_gamma / np.sqrt(bn_var + EPS)
    shift = (bn_beta - bn_mean * scale).astype(np.float32).reshape(CC, 1)
    wsc = (conv_w * scale[:, None, None, None]).astype(np.float32)
    # lhsT layout per tap t = ky*3+kx: w_host[i, t*CC + o] = wsc[o, i, ky, kx]
    w_host = np.ascontiguousarray(
        wsc.transpose(1, 2, 3, 0).reshape(CC, 9 * CC)
    )

    in_maps = []
    for m in range(N_CORES):
        b = m // 2
        s0 = (m % 2) * IMG_PER_CORE
        in_maps.append(
            {
                "u": np.ascontiguousarray(u[b, 0]),
                "v": np.ascontiguousarray(v[b, s0 : s0 + IMG_PER_CORE]),
                "w": w_host,
                "shift": shift,
            }
        )
    return in_maps


def _run(inputs, trace=False):
    nc = _get_program()
    in_maps = _prep_inputs(**inputs)
    res = run_bass_kernel_spmd(
        nc, in_maps, list(range(N_CORES)), trace=trace
    )
    out = np.empty((B, 1, S, CC, H, W), np.float32)
    for m in range(N_CORES):
        b = m // 2
        s0 = (m % 2) * IMG_PER_CORE
        out[b, 0, s0 : s0 + IMG_PER_CORE] = res.results[m]["o"]
    return out, res


def kernel(u, v, conv_w, bn_gamma, bn_beta, bn_mean, bn_var):
    out, _ = _run(
        dict(
            u=u,
            v=v,
            conv_w=conv_w,
            bn_gamma=bn_gamma,
            bn_beta=bn_beta,
            bn_mean=bn_mean,
            bn_var=bn_var,
        )
    )
    return out
